# revision 1
# baseline (speedup 1.0000x reference)
"""Trainium2 Bass kernel for nn_LinearGML2.

Computes out[b, k] = || (x_b - w_k) @ L_k ||_2 for K=256 per-class
lower-triangular matrices L_k (diag = L_diags**2, strict lower = L_lower,
row-major tril order), B=1024, d=512.

Strategy (8 NeuronCores, sharded over classes - 32 classes per core):
  * Host packs, per class, the four 128-row blocks of L_k as dense
    [128, (r+1)*128] bf16 tiles (row-block r only has columns < (r+1)*128
    because L is lower triangular) concatenated into a [128, 1280] tile.
    This skips the 6 all-zero 128x128 blocks: 10/16 of the dense flops.
  * Device: VectorE forms X~^T = X^T - w_k^T (bf16, per-partition scalar
    subtract).  TensorE runs 4 accumulating matmuls per (class, B-chunk)
    with descending free size (512,384,256,128) so PSUM ends up holding
    dist = (x - w_k) @ L_k directly.
  * Epilogue, one pass per PSUM tile split across engines: VectorE
    bn_stats (sum of squares = M2_e + M2_o + 256*(mean_e^2 + mean_o^2))
    for chunks 0-2 (plus chunk 3 on odd classes), ScalarE
    activation(Square, accum_out) for the rest - balances both engines
    just below TensorE's per-class rate.
  * bn_stats decode + sqrt run per 8-class group (overlapped with mains);
    one strided DMA writes the whole [1024, 32] f32 result at the end.
"""

from contextlib import ExitStack

import ml_dtypes
import numpy as np

import concourse.bass as bass  # noqa: F401  (import keeps bass registered)
import concourse.tile as tile
from concourse import bacc, mybir
from concourse._compat import with_exitstack
from concourse.alu_op_type import AluOpType
from concourse.bass_utils import run_bass_kernel_spmd

K_CLASSES = 256
D = 512
B = 1024
N_CORES = 8
KC = K_CLASSES // N_CORES  # classes per core = 32
P = 128
NR = D // P  # row blocks = 4
NCH = B // P  # batch chunks = 8
LOFF = (0, 128, 384, 768)  # col offset of row-block r inside the packed L tile
LT_W = 1280  # 128 + 256 + 384 + 512
KG = 8  # classes per decode/sqrt group

BF16 = ml_dtypes.bfloat16
_BF = mybir.dt.bfloat16
_F32 = mybir.dt.float32
_SQUARE = mybir.ActivationFunctionType.Square
_SQRT = mybir.ActivationFunctionType.Sqrt


def _n_dve(k):
    """Number of trailing chunks whose epilogue runs on VectorE for class k."""
    return 3 + ((k + 1) % 2)


@with_exitstack
def _gml2_kernel(ctx: ExitStack, tc: "tile.TileContext", out, xt, wt, lt):
    nc = tc.nc
    const = ctx.enter_context(tc.tile_pool(name="const", bufs=1))
    lpool = ctx.enter_context(tc.tile_pool(name="lk", bufs=8))
    xpool = ctx.enter_context(tc.tile_pool(name="xtil", bufs=4))
    spool = ctx.enter_context(tc.tile_pool(name="stats", bufs=1))
    gpool = ctx.enter_context(tc.tile_pool(name="grp", bufs=2))
    psum = ctx.enter_context(tc.tile_pool(name="psum", bufs=8, space="PSUM"))

    # per-r tiles so the first prep/matmul only waits on the r=3 transfers.
    # r=3 data first: the first matmul of every (class, chunk) group needs
    # row-block 3.  xt/wt go on the scalar-engine DMA queue (idle early),
    # L tiles on sync.
    xt_sb = [None] * NR
    wt_sb = [None] * NR
    for r in (3, 2, 1, 0):
        xt_sb[r] = const.tile([P, B], _BF, name=f"xt_sb{r}")
        wt_sb[r] = const.tile([P, KC], _F32, name=f"wt_sb{r}")
    Q = B // 4
    for r in (3, 2, 1, 0):  # class-0 critical path on gpsimd (earliest preamble)
        nc.gpsimd.dma_start(wt_sb[r][:, :], wt[r])
        nc.gpsimd.dma_start(xt_sb[r][:, 0:Q], xt[r][:, 0:Q])
    for r in (3, 2, 1, 0):
        nc.scalar.dma_start(xt_sb[r][:, Q : 2 * Q], xt[r][:, Q : 2 * Q])
    for r in (3, 2, 1, 0):
        nc.scalar.dma_start(xt_sb[r][:, 2 * Q : B], xt[r][:, 2 * Q : B])

    sq = spool.tile([P, NCH, KC], _F32)  # per-(chunk, class) sum of squares
    bns = spool.tile([P, 4, KC, 6], _F32)  # bn_stats raw out for chunks 0-3
    outsb = spool.tile([P, NCH, KC], _F32)

    xtils = {}

    def prep(k):
        # X~^T = X^T - w_k^T, per-partition scalar subtract (r=3 first so the
        # leading matmul's weights are ready earliest).  Class 0 is split into
        # halves so its first chunks only wait on the first-half xt DMAs.
        t = xpool.tile([P, NR, B], _BF, tag="xtil")
        halves = (
            ((0, B // 4), (B // 4, B // 2), (B // 2, B)) if k == 0 else ((0, B),)
        )
        for lo, hi in halves:
            for r in (3, 2, 1, 0):
                nc.vector.tensor_scalar_sub(
                    t[:, r, lo:hi], xt_sb[r][:, lo:hi], wt_sb[r][:, k : k + 1]
                )
        xtils[k] = t

    def decode(ks, nch, ch0=0):
        # sum(z^2) = M2_e + M2_o + 256 * (mean_e^2 + mean_o^2) for the class
        # slice `ks`, bns slots 0:nch, written to sq chunks ch0:ch0+nch
        me, m2e = bns[:, 0:nch, ks, 1], bns[:, 0:nch, ks, 2]
        mo, m2o = bns[:, 0:nch, ks, 4], bns[:, 0:nch, ks, 5]
        nk = len(range(*ks.indices(KC)))
        t1f = gpool.tile([P, 4, KG], _F32, tag="t1")
        t2f = gpool.tile([P, 4, KG], _F32, tag="t2")
        t1 = t1f[:, 0:nch, 0:nk]
        t2 = t2f[:, 0:nch, 0:nk]
        nc.vector.tensor_mul(t1, me, me)
        nc.vector.tensor_mul(t2, mo, mo)
        nc.vector.tensor_add(t1, t1, t2)
        nc.vector.tensor_add(t2, m2e, m2o)
        nc.vector.scalar_tensor_tensor(
            sq[:, ch0 : ch0 + nch, ks], t1, float(D // 2), t2,
            AluOpType.mult, AluOpType.add,
        )

    def decode_group(g):
        k0 = g * KG
        klast = k0 + KG - 1
        decode(slice(k0, k0 + KG, 2), 4, ch0=4)  # even classes: chunks 4-7
        decode(slice(k0 + 1, k0 + KG, 2), 3, ch0=5)  # odd classes: chunks 5-7
        nc.scalar.activation(
            outsb[:, :, k0 : k0 + KG], sq[:, :, k0 : k0 + KG], _SQRT
        )
        nc.sync.dma_start(
            out.rearrange("c p k -> p c k")[:, :, k0 : k0 + KG],
            outsb[:, :, k0 : k0 + KG],
        )

    prep(0)
    for k in range(KC):
        lk = [None] * NR  # per-r tiles, r=3 first (needed by the leading matmul)
        for r in (3, 2, 1, 0):
            n = (r + 1) * P
            lkr = lpool.tile([P, n], _BF, tag=f"lk{r}", name=f"lk{r}")
            nc.sync.dma_start(lkr[:, :], lt[k, :, LOFF[r] : LOFF[r] + n])
            lk[r] = lkr
        if k + 1 < KC:
            prep(k + 1)  # VectorE runs one class ahead of TensorE
        xtil = xtils.pop(k)
        ndve = _n_dve(k)
        for ch in range(NCH):
            pt = psum.tile([P, 512], _F32, tag="pt")
            for r in (3, 2, 1, 0):
                n = (r + 1) * P
                nc.tensor.matmul(
                    pt[:, 0:n],
                    xtil[:, r, ch * P : (ch + 1) * P],  # lhsT [K=128, M=128]
                    lk[r][:, 0:n],  # rhs [K=128, N=n]
                    start=(r == 3),
                    stop=(r == 0),
                )
            on_v = ch >= NCH - ndve  # V takes late chunks (prep runs early)
            slot = ch - (NCH - ndve)
            if on_v:
                nc.vector.bn_stats(bns[:, slot, k, :], pt[:, :])
            else:
                nc.scalar.activation(
                    pt[:, :], pt[:, :], _SQUARE, accum_out=sq[:, ch, k : k + 1]
                )
        if k % KG == KG - 1:
            decode_group(k // KG)



_CACHE: dict = {}


def build_nc():
    if "nc" in _CACHE:
        return _CACHE["nc"]
    nc = bacc.Bacc("TRN2", target_bir_lowering=False, debug=False, num_devices=N_CORES)
    xt = nc.dram_tensor("xt", [NR, P, B], _BF, kind="ExternalInput").ap()
    wt = nc.dram_tensor("wt", [NR, P, KC], _F32, kind="ExternalInput").ap()
    lt = nc.dram_tensor("lt", [KC, P, LT_W], _BF, kind="ExternalInput").ap()
    out = nc.dram_tensor("out", [NCH, P, KC], _F32, kind="ExternalOutput").ap()
    with tile.TileContext(nc) as tc:
        _gml2_kernel(tc, out, xt, wt, lt)
    nc.compile()
    _CACHE["nc"] = nc
    return nc


def host_prep(inputs, weight, L_diags, L_lower):
    """Pure layout/dtype transforms of the inputs (no math beyond L assembly)."""
    x = np.asarray(inputs, dtype=np.float32)
    w = np.asarray(weight, dtype=np.float32).reshape(K_CLASSES, D)
    ld = np.asarray(L_diags, dtype=np.float32)
    ll = np.asarray(L_lower, dtype=np.float32)

    lmat = np.zeros((K_CLASSES, D, D), dtype=np.float32)
    ri, ci = np.tril_indices(D, k=-1)
    lmat[:, ri, ci] = ll
    dd = np.arange(D)
    lmat[:, dd, dd] = ld * ld

    lt = np.concatenate(
        [lmat[:, r * P : (r + 1) * P, : (r + 1) * P] for r in range(NR)], axis=2
    ).astype(BF16)  # [K, 128, 1280]
    xt = np.ascontiguousarray(x.T).reshape(NR, P, B).astype(BF16)
    return xt, w, lt


def make_in_maps(xt, w, lt):
    in_maps = []
    for c in range(N_CORES):
        sl = slice(c * KC, (c + 1) * KC)
        wt = np.ascontiguousarray(w[sl].T).reshape(NR, P, KC).astype(np.float32)
        in_maps.append({"xt": xt, "wt": wt, "lt": np.ascontiguousarray(lt[sl])})
    return in_maps


def kernel(inputs, weight, L_diags, L_lower, **run_kwargs):
    xt, w, lt = host_prep(inputs, weight, L_diags, L_lower)
    nc = build_nc()
    in_maps = make_in_maps(xt, w, lt)
    res = run_bass_kernel_spmd(nc, in_maps, core_ids=list(range(N_CORES)), **run_kwargs)
    out = np.empty((B, K_CLASSES), dtype=np.float32)
    for c in range(N_CORES):
        out[:, c * KC : (c + 1) * KC] = (
            np.asarray(res.results[c]["out"]).astype(np.float32).reshape(B, KC)
        )
    if run_kwargs:
        _CACHE["last_result"] = res
    return out

